# revision 15
# baseline (speedup 1.0000x reference)
"""KAN (B-spline) network kernel for 8 Trainium2 NeuronCores.

Strategy:
- Data-parallel over batch: 8192 rows -> 1024 per core; weights replicated
  (NEFF Const tensors), float32r matmuls (1 cyc/row, ~17 effective mantissa
  bits -- enough for the truncated-power cancellation).
- Spline term, truncated power form: u = 2.5x + 8 clamped to <=16,
  sum_g N3(u-g)*D[g] == sum_{s=0..15} beta_s relu(u-s)^3 (slot 16 == 0).
  Slots processed in groups of 4 (2048-col ops amortize instruction
  overhead).  Pattern A (ACT-first): r = Relu(uc-s) on ACT, sq = r*r,
  cu = sq*r as plain muls on Pool/DVE.  Pattern B (DVE-first):
  d = (ua min 16) - s fused on DVE, sq = Square(d) on ACT (sign-wrong for
  d<0 but fixed by cube), cu = (d max 0)*sq fused stt on DVE.
- Base term mish = h*tanh(softplus(h)) with softplus(h) = relu(h) +
  ln2*exp(-A|h| - B|h|^2) (fitted, max err 3e-3; mish err <= 8e-3).
  Uses only Abs/Exp/Relu/Tanh -- together with Square and softmax's
  Exp/Copy all in the exp_and_others ACT table set, so the main phase
  needs no table switches; softmax's 8 Ln ops are batched at the end.
- Two batch tiles of 512, emitted layer-interleaved for cross-tile
  pipelining of PE vs ACT/DVE/Pool.
"""
import sys
import os

sys.path.insert(0, '/opt/trn_rl_repo')

import numpy as np
from contextlib import ExitStack

import concourse.bass as bass
import concourse.bacc as bacc
import concourse.tile as tile
from concourse import mybir
from concourse.bass_utils import run_bass_kernel_spmd

F32 = mybir.dt.float32
F32R = mybir.dt.float32r
AF = mybir.ActivationFunctionType
ALU = mybir.AluOpType

N_CORES = 8
B_TOTAL = 8192
B_CORE = B_TOTAL // N_CORES     # 1024
BT = 512                        # batch tile (free dim)
NBT = B_CORE // BT              # 2
K_ORD, GRID = 3, 10
LO, HI = -2.0, 2.0
H = (HI - LO) / GRID            # 0.4
NC_B = GRID + K_ORD             # 13 basis functions
NS = 16                         # truncated-power slots s = 0..15
NJ1 = 8                         # L1 two-packed slot pairs
USC, UOF = 1.0 / H, K_ORD - LO / H   # u = 2.5x + 8
MISH_A, MISH_B = 0.749496, 0.045652  # ln(1+e^-y) ~= ln2 e^(-Ay-By^2)
LNLN2 = float(np.log(np.log(2.0)))

_CACHE = {}
LAST_RES = None


def _beta(coef, sp):
    """beta[i,s,o]: sum_g D[i,g,o] N3(u-g) = sum_s beta[i,s,o] relu(u-s)^3
    on [0,16]; slot 16 dropped (relu(u-16)=0 for u<=16)."""
    D = (coef * sp[..., None]).astype(np.float64)          # (in, out, 13)
    c = np.array([1.0, -4.0, 6.0, -4.0, 1.0]) / 6.0
    fin, fout = D.shape[0], D.shape[1]
    beta = np.zeros((fin, 17, fout))
    for g in range(NC_B):
        for r in range(5):
            beta[:, g + r, :] += c[r] * D[:, :, g]
    return beta[:, :NS, :].astype(np.float32)              # (in, 16, out)


def _build(weights):
    nc = bacc.Bacc("TRN2", target_bir_lowering=False, debug=False,
                   num_devices=N_CORES)
    xT = nc.dram_tensor("xT", [49, B_CORE], F32, kind="ExternalInput")
    out_d = nc.dram_tensor("out", [B_CORE, 10], F32, kind="ExternalOutput")

    b1 = weights['b1']; b2 = weights['b2']; b3 = weights['b3']
    beta1 = _beta(weights['coef1'], weights['sp1'])    # (49, 16, 256)
    beta2 = _beta(weights['coef2'], weights['sp2'])    # (256, 16, 256)
    beta3 = _beta(weights['coef3'], weights['sp3'])    # (256, 16, 10)

    # L1 two-pack: partition p<49 -> (i=p, s=2j), p>=49 -> (i=p-49, s=2j+1)
    e1 = np.zeros((98, NJ1, 256), np.float32)
    negs1 = np.zeros((98, NJ1), np.float32)
    for j in range(NJ1):
        e1[:49, j, :] = beta1[:, 2 * j, :]
        e1[49:, j, :] = beta1[:, 2 * j + 1, :]
        negs1[:49, j] = -(2 * j)
        negs1[49:, j] = -(2 * j + 1)

    consts = {
        'e1': e1.reshape(98, NJ1 * 256),
        'negs1': negs1,
        'negs128': np.tile(-np.arange(NS, dtype=np.float32), (128, 1)),
        'lnln2': np.full((128, 1), LNLN2, np.float32),
        'e2': np.ascontiguousarray(beta2.reshape(2, 128, NS * 256)),
        'e3': np.ascontiguousarray(beta3.reshape(2, 128, NS * 10)),
        'sb1': weights['sb1'].astype(np.float32),
        'sb2': weights['sb2'].astype(np.float32),
        'sb3': weights['sb3'].astype(np.float32),
        'bias1': b1.reshape(2, 128, 1).astype(np.float32),
        'bias2': b2.reshape(2, 128, 1).astype(np.float32),
        'bias3': b3.reshape(10, 1).astype(np.float32),
        'ubias1': (USC * b1 + UOF).reshape(2, 128, 1).astype(np.float32),
        'ubias2': (USC * b2 + UOF).reshape(2, 128, 1).astype(np.float32),
        'eye': np.eye(128, dtype=np.float32),
    }
    dts = {k: nc.inline_tensor(v, name=k) for k, v in consts.items()}

    with tile.TileContext(nc) as tc, ExitStack() as ctx:
        wpool = ctx.enter_context(tc.tile_pool(name="w", bufs=1))
        e1t = wpool.tile([98, NJ1 * 256], F32R)
        nc.sync.dma_start(e1t[:], dts['e1'].ap().bitcast(F32R))
        negs1t = wpool.tile([98, NJ1], F32)
        nc.sync.dma_start(negs1t[:], dts['negs1'].ap())
        negs128t = wpool.tile([128, NS], F32)
        nc.sync.dma_start(negs128t[:], dts['negs128'].ap())
        lnln2t = wpool.tile([128, 1], F32)
        nc.sync.dma_start(lnln2t[:], dts['lnln2'].ap())
        e2t = [wpool.tile([128, NS * 256], F32R, tag=f"e2_{ic}", name=f"e2_{ic}")
               for ic in range(2)]
        for ic in range(2):
            nc.sync.dma_start(e2t[ic][:], dts['e2'].ap().bitcast(F32R)[ic])
        e3t = [wpool.tile([128, NS * 10], F32R, tag=f"e3_{ic}", name=f"e3_{ic}")
               for ic in range(2)]
        for ic in range(2):
            nc.sync.dma_start(e3t[ic][:], dts['e3'].ap().bitcast(F32R)[ic])
        sb1t = wpool.tile([49, 256], F32R)
        nc.sync.dma_start(sb1t[:], dts['sb1'].ap().bitcast(F32R))
        sb2t = [wpool.tile([128, 256], F32R, tag=f"sb2_{ic}", name=f"sb2_{ic}")
                for ic in range(2)]
        for ic in range(2):
            nc.sync.dma_start(sb2t[ic][:],
                              dts['sb2'].ap().bitcast(F32R)[ic * 128:(ic + 1) * 128, :])
        sb3t = [wpool.tile([128, 10], F32R, tag=f"sb3_{ic}", name=f"sb3_{ic}")
                for ic in range(2)]
        for ic in range(2):
            nc.sync.dma_start(sb3t[ic][:],
                              dts['sb3'].ap().bitcast(F32R)[ic * 128:(ic + 1) * 128, :])
        bias1t, ubias1t, bias2t, ubias2t = [], [], [], []
        for oc in range(2):
            for lst, key in [(bias1t, 'bias1'), (ubias1t, 'ubias1'),
                             (bias2t, 'bias2'), (ubias2t, 'ubias2')]:
                t = wpool.tile([128, 1], F32, tag=f"{key}_{oc}", name=f"{key}_{oc}")
                nc.sync.dma_start(t[:], dts[key].ap()[oc])
                lst.append(t)
        bias3t = wpool.tile([10, 1], F32)
        nc.sync.dma_start(bias3t[:], dts['bias3'].ap())
        eyet = wpool.tile([128, 128], F32)
        nc.sync.dma_start(eyet[:], dts['eye'].ap())

        io = ctx.enter_context(tc.tile_pool(name="io", bufs=2))
        nar = ctx.enter_context(tc.tile_pool(name="nar", bufs=2))
        slt = ctx.enter_context(tc.tile_pool(name="slt", bufs=3))
        ps = ctx.enter_context(tc.tile_pool(name="ps", bufs=1, space="PSUM"))
        sm = ctx.enter_context(tc.tile_pool(name="sm", bufs=2))

        # global round-robin engine schedulers for sq and pattern-A cubes
        sq_cycle = ['P', 'A', 'P', 'V', 'A', 'P', 'P', 'A']
        cu_cycle = ['V', 'P']
        cnt = {'sq': 0, 'cu': 0}

        def slot_group_chain(ua, uc, parts, nslot, blk, negs_of, emit_mms):
            """Process nslot slots in groups of 4; emit_mms(g, cg) is
            called right after each group's cube tile so PE consumes it
            immediately (short cg lifetime, early PE start)."""
            for g in range(nslot // 4):
                pat_a = (g % 4 != 3)  # 3:1 A:B ratio; L1 (2 grps) all A
                dg = slt.tile([128, 4 * BT], F32, tag="dg", name=f"d{blk}_{g}")
                for k in range(4):
                    s = 4 * g + k
                    sl = dg[0:parts, k * BT:(k + 1) * BT]
                    if pat_a:
                        nc.scalar.activation(sl, uc, AF.Relu, bias=negs_of(s))
                    else:
                        nc.vector.tensor_scalar(sl, ua, 16.0, float(s),
                                                ALU.min, ALU.subtract)
                qg = slt.tile([128, 4 * BT], F32, tag="qg", name=f"q{blk}_{g}")
                if pat_a:
                    e = sq_cycle[cnt['sq'] % len(sq_cycle)]; cnt['sq'] += 1
                    if e == 'P':
                        nc.gpsimd.tensor_mul(qg[0:parts, :], dg[0:parts, :],
                                             dg[0:parts, :])
                    elif e == 'V':
                        nc.vector.tensor_mul(qg[0:parts, :], dg[0:parts, :],
                                             dg[0:parts, :])
                    else:
                        nc.scalar.activation(qg[0:parts, :], dg[0:parts, :],
                                             AF.Square)
                else:
                    nc.scalar.activation(qg[0:parts, :], dg[0:parts, :],
                                         AF.Square)
                cg = slt.tile([128, 4 * BT], F32R, tag="cg", name=f"c{blk}_{g}")
                if pat_a:
                    e = cu_cycle[cnt['cu'] % len(cu_cycle)]; cnt['cu'] += 1
                    eng = nc.vector if e == 'V' else nc.gpsimd
                    eng.tensor_mul(cg[0:parts, :], qg[0:parts, :],
                                   dg[0:parts, :])
                else:
                    nc.vector.scalar_tensor_tensor(cg[0:parts, :],
                                                   dg[0:parts, :], 0.0,
                                                   qg[0:parts, :],
                                                   ALU.max, ALU.mult)
                emit_mms(g, cg)

        def mish_of(src, bias_ap, parts, blk):
            """m = h*tanh(softplus(h)), h = src+bias, via
            softplus(h) = relu(h) + ln2*exp(-A|h| - B|h|^2)."""
            def nt(tag, dt=F32):
                return nar.tile([128, BT], dt, tag=tag,
                                name=f"{tag}{blk}")[0:parts, :]
            bias = 0.0 if bias_ap is None else bias_ap
            y = nt("my")
            nc.scalar.activation(y, src, AF.Abs, bias=bias)
            t = nt("mt")
            nc.vector.tensor_scalar(t, y, MISH_B, MISH_A, ALU.mult, ALU.add)
            aa = nt("ma")
            nc.vector.scalar_tensor_tensor(aa, y, -1.0, t, ALU.mult, ALU.mult)
            e = nt("me")
            nc.scalar.activation(e, aa, AF.Exp, bias=lnln2t[0:parts, :])
            r = nt("mr")
            nc.scalar.activation(r, src, AF.Relu, bias=bias)
            sp = nt("msp")
            nc.vector.tensor_add(sp, r, e)
            w = nt("mw")
            nc.scalar.activation(w, sp, AF.Tanh)
            m = nar.tile([128, BT], F32R, tag="mm", bufs=4,
                         name=f"mm{blk}")[0:parts, :]
            if bias_ap is None:
                nc.vector.tensor_mul(m, src, w)
            else:
                nc.vector.scalar_tensor_tensor(m, src, bias_ap, w,
                                               ALU.add, ALU.mult)
            return m

        lg = []
        ps1s, ps2s, ps3s = {}, {}, {}
        xts = {}

        for bt in range(NBT):
            bsl = slice(bt * BT, (bt + 1) * BT)
            xt = io.tile([98, BT], F32, tag="xt", name=f"xt{bt}")
            nc.sync.dma_start(xt[0:49, :], xT.ap()[:, bsl])
            nc.sync.dma_start(xt[49:98, :], xT.ap()[:, bsl])
            xts[bt] = xt

        # ---- L1 ----
        for bt in range(NBT):
            xt = xts[bt]
            ua = nar.tile([128, BT], F32, tag="ua", bufs=3,
                          name=f"ua1_{bt}")[0:98, :]
            nc.vector.tensor_scalar(ua, xt[:], USC, UOF, ALU.mult, ALU.add)
            uc = nar.tile([128, BT], F32, tag="uc", bufs=3,
                          name=f"uc1_{bt}")[0:98, :]
            nc.vector.tensor_scalar(uc, ua, 16.0, None, ALU.min)
            mish1 = mish_of(xt[0:49, :], None, 49, f"m1_{bt}")

            p1 = [ps.tile([128, BT], F32, tag=f"ps1_{oc}", bufs=1,
                          name=f"ps1_{oc}_{bt}") for oc in range(2)]
            ps1s[bt] = p1
            for oc in range(2):
                nc.tensor.matmul(p1[oc][:], sb1t[:, oc * 128:(oc + 1) * 128],
                                 mish1, start=True, stop=False)
            def l1_mms(g, cg):
                for k in range(4):
                    j = 4 * g + k
                    for oc in range(2):
                        nc.tensor.matmul(
                            p1[oc][:],
                            e1t[:, j * 256 + oc * 128: j * 256 + (oc + 1) * 128],
                            cg[0:98, k * BT:(k + 1) * BT],
                            start=False, stop=(j == NJ1 - 1))
            slot_group_chain(ua, uc, 98, NJ1, f"L1t{bt}",
                             lambda s: negs1t[:, s:s + 1], l1_mms)

        def mid_layer(bt, srcs, et, sbt, biast, ubiast, nout, blk):
            if nout == 256:
                po = [ps.tile([128, BT], F32, tag=f"ps2_{oc}", bufs=2,
                              name=f"ps2_{oc}_{bt}") for oc in range(2)]
            else:
                po = [ps.tile([10, BT], F32, tag="ps3", name=f"ps3_{bt}")]
            mishes, uas, ucs = [], [], []
            for ic in range(2):
                mishes.append(mish_of(srcs[ic][:], biast[ic][:], 128,
                                      f"m{blk}_{ic}"))
                ua = nar.tile([128, BT], F32, tag="ua", bufs=3,
                              name=f"ua{blk}_{ic}")
                nc.vector.tensor_scalar(ua[:], srcs[ic][:], USC, ubiast[ic][:],
                                        ALU.mult, ALU.add)
                uc = nar.tile([128, BT], F32, tag="uc", bufs=3,
                              name=f"uc{blk}_{ic}")
                nc.vector.tensor_scalar(uc[:], ua[:], 16.0, None, ALU.min)
                uas.append(ua[:]); ucs.append(uc[:])
            for ic in range(2):
                if nout == 256:
                    for k, p in enumerate(po):
                        nc.tensor.matmul(p[:], sbt[ic][:, k * 128:(k + 1) * 128],
                                         mishes[ic], start=(ic == 0),
                                         stop=False)
                else:
                    nc.tensor.matmul(po[0][:], sbt[ic][:], mishes[ic],
                                     start=(ic == 0), stop=False)
            for ic in range(2):
                last = (ic == 1)

                def mid_mms(g, cg, ic=ic, last=last):
                    for k in range(4):
                        s = 4 * g + k
                        stop = last and s == NS - 1
                        if nout == 256:
                            for kk, p in enumerate(po):
                                nc.tensor.matmul(
                                    p[:],
                                    et[ic][:, s * 256 + kk * 128: s * 256 + (kk + 1) * 128],
                                    cg[0:128, k * BT:(k + 1) * BT],
                                    start=False, stop=stop and kk == 1)
                        else:
                            nc.tensor.matmul(
                                po[0][:], et[ic][:, s * 10:(s + 1) * 10],
                                cg[0:128, k * BT:(k + 1) * BT],
                                start=False, stop=stop)
                slot_group_chain(uas[ic], ucs[ic], 128, NS, f"{blk}i{ic}",
                                 lambda s: negs128t[:, s:s + 1], mid_mms)
            return po

        for bt in range(NBT):
            ps2s[bt] = mid_layer(bt, ps1s[bt], e2t, sb2t, bias1t, ubias1t,
                                 256, f"L2t{bt}")
        for bt in range(NBT):
            ps3s[bt] = mid_layer(bt, ps2s[bt], e3t, sb3t, bias2t, ubias2t,
                                 10, f"L3t{bt}")
            l = sm.tile([10, BT], F32, tag=f"lg{bt}", bufs=1, name=f"lg{bt}")
            nc.vector.tensor_scalar(l[:], ps3s[bt][0][:], bias3t[:], None,
                                    ALU.add)
            lg.append(l)

        # ---- log_softmax, stage-batched so the 8 Ln ops cost a single
        # table switch at the very end ----
        chunks = [(bt, c4) for bt in range(NBT) for c4 in range(BT // 128)]
        ts, nmxs = [], []
        for i, (bt, c4) in enumerate(chunks):
            tp = ps.tile([128, 10], F32, tag="tp", bufs=1, name="tp")
            nc.tensor.transpose(tp[:], lg[bt][:, c4 * 128:(c4 + 1) * 128],
                                eyet[0:10, 0:10])
            t = sm.tile([128, 10], F32, tag="smt", bufs=8, name=f"t{i}")
            nc.scalar.activation(t[:], tp[:], AF.Copy)
            ts.append(t)
        for i in range(8):
            mx = sm.tile([128, 1], F32, tag="mx", bufs=2, name="mx")
            nc.vector.reduce_max(mx[:], ts[i][:], axis=mybir.AxisListType.X)
            nmx = sm.tile([128, 1], F32, tag="nmx", bufs=8, name=f"nmx{i}")
            nc.vector.tensor_scalar(nmx[:], mx[:], -1.0, None, ALU.mult)
            nmxs.append(nmx)
        exs = []
        for i in range(8):
            ex = sm.tile([128, 10], F32, tag="ex", bufs=8, name=f"ex{i}")
            nc.scalar.activation(ex[:], ts[i][:], AF.Exp, bias=nmxs[i][:])
            exs.append(ex)
        ssums = []
        for i in range(8):
            ssum = sm.tile([128, 1], F32, tag="ssum", bufs=8, name=f"ss{i}")
            nc.vector.reduce_sum(ssum[:], exs[i][:], axis=mybir.AxisListType.X)
            ssums.append(ssum)
        lnss = []
        for i in range(8):
            lns = sm.tile([128, 1], F32, tag="lns", bufs=8, name=f"ln{i}")
            nc.scalar.activation(lns[:], ssums[i][:], AF.Ln)
            lnss.append(lns)
        for i, (bt, c4) in enumerate(chunks):
            off = sm.tile([128, 1], F32, tag="off", bufs=2, name="off")
            nc.vector.tensor_sub(off[:], nmxs[i][:], lnss[i][:])
            res = sm.tile([128, 10], F32, tag="res", bufs=2, name="res")
            nc.vector.tensor_scalar(res[:], ts[i][:], off[:], None, ALU.add)
            nc.sync.dma_start(
                out_d.ap()[bt * BT + c4 * 128: bt * BT + (c4 + 1) * 128, :],
                res[:])

    nc.finalize()
    return nc


def kernel(**inputs):
    x = np.asarray(inputs['x'], np.float32)
    B = x.shape[0]
    pooled = x.reshape(B, 7, 4, 7, 4).mean(axis=(2, 4)).reshape(B, 49)
    xT = np.ascontiguousarray(pooled.T)                   # (49, 8192)

    key = 'nc'
    if key not in _CACHE:
        _CACHE[key] = _build(inputs)
    nc = _CACHE[key]

    in_maps = [{"xT": np.ascontiguousarray(
        xT[:, c * B_CORE:(c + 1) * B_CORE])} for c in range(N_CORES)]
    res = run_bass_kernel_spmd(nc, in_maps, core_ids=list(range(N_CORES)))
    global LAST_RES
    LAST_RES = res
    out = np.concatenate([res.results[c]["out"] for c in range(N_CORES)], axis=0)
    return out.astype(np.float32)


if __name__ == "__main__":
    d = np.load('/root/problem/ref_data.npz')
    inputs = {k: d[k] for k in d.files if k != 'expected'}
    out = kernel(**inputs)
    exp = d['expected']
    err = np.abs(out - exp).max()
    rel = err / np.abs(exp).max()
    print(f"maxabs={err:.6g} rel={rel:.3g}")
